# revision 19
# baseline (speedup 1.0000x reference)
"""Causal self-attention (B=4, T=2048, C=1024, H=16) on 8 Trainium2 NeuronCores.

Core index = 2*batch + head_group: each core owns one batch element and 8 of
the 16 heads (tensor-parallel split of c_attn output dim / c_proj input dim).
Each core emits a partial projection out^T [C, T]; the host sums the two
head-group partials per batch and adds the bias terms.

Per-core pipeline (Tile-scheduled, phases overlap via data deps):
  A/B (4 t-stripes of 512):
     xT stripe via PE-transpose (fp32) -> f32r
     qkT[co, stripe] = W_qk^T x^T   (f32r matmuls, full PE rate)
     v[stripe] = x @ W_v natural [t, d] layout -> fp16 (+ ones column)
  C (per head h, per 512-wide i-chunk ic):
     S^T[j, i] = k_h^T q_h   f32r, psum groups of 2 j-tiles [128, 2, 512]
     P = exp(S^T / 8)        one ACT op per group -> fp16
     causal mask on diagonal groups (DVE, precomputed mask tiles)
     U'^T [65, i] (+)= v'^T P^T  over j-tiles (fp16; ones column => rowsum)
     yT[hd, i] = U'^T[:64] * bcast(1/rowsum)  (gpsimd partition_broadcast
                 + DVE reciprocal + multiply) -> f32r
  D: out^T = W_p^T yT  (f32r)
"""

import numpy as np

import concourse.bass as bass
import concourse.mybir as mybir
import concourse.tile as tile
from concourse import bacc, bass_utils

B, T, C, H = 4, 2048, 1024, 16
HD = C // H          # 64 head dim
N_CORES = 8
HG = H // 2          # 8 heads per core
CL = HG * HD         # 512 local width of q/k/v
TT = T // 128        # 16 t-tiles
CB = C // 128        # 8 c-tiles
DB = CL // 128       # 4 local-hd tiles
NS = 4               # t-stripes
SW = T // NS         # stripe width (512)
NIC = T // 512       # i-chunks (4)

f32 = mybir.dt.float32
f32r = mybir.dt.float32r
f16 = mybir.dt.float16

_PROG_CACHE = {}


def _emit(tc, aps):
    nc = tc.nc
    Exp = mybir.ActivationFunctionType.Exp
    Copy = mybir.ActivationFunctionType.Copy

    x_ap = aps["x"]
    wqk_ap = aps["wqk"]
    wv_ap = aps["wv"]
    wp_ap = aps["wp"]
    bqk_ap = aps["bqk"]
    masks_ap = aps["masks"]
    id32_ap = aps["id32"]
    outT_ap = aps["outT"]

    from contextlib import ExitStack

    with ExitStack() as outer:
        const = outer.enter_context(tc.tile_pool(name="const", bufs=1))
        p_qkT = outer.enter_context(tc.tile_pool(name="qkT", bufs=1))
        p_v = outer.enter_context(tc.tile_pool(name="vv", bufs=1))
        p_yT = outer.enter_context(tc.tile_pool(name="yT", bufs=1))

        id32 = const.tile([128, 128], f32)
        nc.sync.dma_start(id32[:], id32_ap)
        masks = const.tile([128, 4, 512], f16)
        nc.sync.dma_start(masks[:], masks_ap)
        bqk = const.tile([128, CB], f32)
        nc.sync.dma_start(bqk[:], bqk_ap.rearrange("co p -> p co"))

        # per-(co, stripe) qkT tiles for fine-grained deps (attention can
        # start on a head as soon as its stripes are produced)
        qkT = {}
        for co in range(CB):
            for s in range(NS):
                qkT[(co, s)] = p_qkT.tile([128, SW], f32r, tag=f"qkT_{co}_{s}", name=f"qkT_{co}_{s}")
        # per-jt v' tiles [j-part, head, d|ones]
        VW = HD + 1  # v' = [v(64) | ones]: U rows 0..63, rowsum at row 64
        vv = {}
        for jt in range(TT):
            vv[jt] = p_v.tile([128, HG, VW], f16, tag=f"vv_{jt}", name=f"vv_{jt}")
            nc.vector.memset(vv[jt][:, :, HD : HD + 1], 1.0)
        yT = p_yT.tile([128, DB, T], f32r)

        def qk_slice(co, lo, width):
            """AP into qkT for c'-tile co, t range [lo, lo+width) (within one stripe)."""
            s = lo // SW
            off = lo - s * SW
            return qkT[(co, s)][:, off : off + width]

        with ExitStack() as s_ab:
            p_xload = s_ab.enter_context(tc.tile_pool(name="xload", bufs=2))
            p_wst = s_ab.enter_context(tc.tile_pool(name="wst", bufs=2))
            p_wqk = s_ab.enter_context(tc.tile_pool(name="wqk", bufs=2))
            p_wv = s_ab.enter_context(tc.tile_pool(name="wv", bufs=1))
            p_xT = s_ab.enter_context(tc.tile_pool(name="xT", bufs=1))
            ps_ab = ExitStack()
            ps_mm = ps_ab.enter_context(tc.tile_pool(name="ps_mm", bufs=2, space="PSUM"))
            ps_tr = ps_ab.enter_context(tc.tile_pool(name="ps_tr", bufs=2, space="PSUM"))

            # round weights to f32r once
            wv_r = p_wv.tile([128, CB, CL], f32r)
            for cb in range(CB):
                wt = p_wst.tile([128, CL], f32, tag="wst")
                nc.sync.dma_start(wt[:], wv_ap[cb * 128 : (cb + 1) * 128, :])
                nc.vector.tensor_copy(wv_r[:, cb, :], wt[:])

            for s in range(NS):
                xT = p_xT.tile([128, CB, SW], f32r, tag="xT")
                for u in range(SW // 128):
                    tt = s * (SW // 128) + u
                    xt = p_xload.tile([128, C], f32, tag="xload")
                    nc.sync.dma_start(xt[:], x_ap[tt * 128 : (tt + 1) * 128, :])
                    for cb in range(CB):
                        pst = ps_tr.tile([128, 128], f32, tag="tr")
                        nc.tensor.transpose(pst[:], xt[:, cb * 128 : (cb + 1) * 128], id32[:])
                        nc.vector.tensor_copy(xT[:, cb, u * 128 : (u + 1) * 128], pst[:])
                # qk for this stripe (wqk streamed per stripe)
                for co in range(CB):
                    wq_r = p_wqk.tile([128, CB, 128], f32r, tag="wqk")
                    wt = p_wst.tile([128, CB, 128], f32, tag="wst")
                    nc.sync.dma_start(wt[:], wqk_ap[co].rearrange("(cb p) q -> p cb q", p=128))
                    nc.vector.tensor_copy(wq_r[:], wt[:])
                    ps = ps_mm.tile([128, SW], f32, tag="mm")
                    for cb in range(CB):
                        nc.tensor.matmul(
                            ps[:],
                            wq_r[:, cb, :],
                            xT[:, cb, :],
                            start=(cb == 0),
                            stop=(cb == CB - 1),
                        )
                    nc.vector.tensor_scalar_add(qkT[(co, s)][:], ps[:], bqk[:, co : co + 1])
                # v for this stripe
                for u in range(SW // 128):
                    tt = s * (SW // 128) + u
                    ps = ps_mm.tile([128, CL], f32, tag="mm")
                    for cb in range(CB):
                        nc.tensor.matmul(
                            ps[:],
                            xT[:, cb, u * 128 : (u + 1) * 128],
                            wv_r[:, cb, :],
                            start=(cb == 0),
                            stop=(cb == CB - 1),
                        )
                    nc.vector.tensor_copy(
                        vv[tt][:, :, 0:HD], ps.rearrange("p (h d) -> p h d", d=HD)
                    )

            ps_ab.close()  # free A/B psum banks (per-tile release deps)

            # ---- attention (emitted while AB sbuf pools still open: disjoint
            # addrs, fine-grained deps let it overlap the tail of A/B) --------
            with ExitStack() as s_c:
                p_p = s_c.enter_context(tc.tile_pool(name="pp", bufs=10))
                p_usb = s_c.enter_context(tc.tile_pool(name="usb", bufs=2))
                p_rb = s_c.enter_context(tc.tile_pool(name="rb", bufs=2))
                ps_sc = s_c.enter_context(tc.tile_pool(name="ps_sc", bufs=3, space="PSUM"))
                ps_u = s_c.enter_context(tc.tile_pool(name="ps_u", bufs=2, space="PSUM"))

                for h in range(HG):
                    poff = 64 * (h % 2)
                    co_q = h // 2
                    co_k = 4 + h // 2
                    for ic in range(NIC):
                        n_jt = 4 * (ic + 1)
                        n_g = n_jt // 2
                        ptiles = []
                        for g in range(n_g):
                            psg = ps_sc.tile([128, 2, 512], f32, tag="sc")
                            for ss in range(2):
                                jt = 2 * g + ss
                                nc.tensor.matmul(
                                    psg[:, ss, :],
                                    qk_slice(co_k, jt * 128, 128)[poff : poff + 64, :],
                                    qk_slice(co_q, ic * 512, 512)[poff : poff + 64, :],
                                    start=True,
                                    stop=True,
                                )
                            pt = p_p.tile([128, 2, 512], f16, tag="p")
                            nc.scalar.activation(pt[:], psg[:], Exp, scale=1.0 / np.sqrt(HD))
                            if 2 * g + 1 >= 4 * ic:  # group contains diagonal blocks
                                m0 = 2 * g - 4 * ic
                                nc.vector.tensor_mul(pt[:], pt[:], masks[:, m0 : m0 + 2, :])
                            ptiles.append(pt)
                        up = ps_u.tile([HD + 1, 512], f32, tag="u")
                        for jt in range(n_jt):
                            nc.tensor.matmul(
                                up[:],
                                vv[jt][:, h, :],
                                ptiles[jt // 2][:, jt % 2, :],
                                start=(jt == 0),
                                stop=(jt == n_jt - 1),
                            )
                        usb = p_usb.tile([HD, 512], f32, tag="usb")
                        nc.scalar.activation(usb[:], up[0:HD, :], Copy)
                        rs = p_rb.tile([1, 512], f32, tag="rs")
                        nc.scalar.activation(rs[:], up[HD : HD + 1, :], Copy)
                        rb = p_rb.tile([HD, 512], f32, tag="rb")
                        nc.gpsimd.partition_broadcast(rb[:], rs[0:1, :], channels=HD)
                        nc.vector.reciprocal(rb[:], rb[:])
                        nc.vector.tensor_mul(
                            yT[poff : poff + HD, h // 2, ic * 512 : (ic + 1) * 512],
                            usb[:],
                            rb[:],
                        )

        # ---- projection ------------------------------------------------------
        with ExitStack() as s_d:
            p_wp = s_d.enter_context(tc.tile_pool(name="wp", bufs=1))
            p_ost = s_d.enter_context(tc.tile_pool(name="ost", bufs=4))
            p_wpst = s_d.enter_context(tc.tile_pool(name="wpst", bufs=2))
            ps_pj = s_d.enter_context(tc.tile_pool(name="ps_pj", bufs=2, space="PSUM"))

            wp_r = p_wp.tile([128, DB, C], f32r)
            for db in range(DB):
                wt = p_wpst.tile([128, C], f32, tag="wpst")
                nc.sync.dma_start(wt[:], wp_ap[db * 128 : (db + 1) * 128, :])
                nc.vector.tensor_copy(wp_r[:, db, :], wt[:])

            for co in range(CB):
                for tn in range(4):
                    ps = ps_pj.tile([128, 512], f32, tag="pj")
                    for db in range(DB):
                        nc.tensor.matmul(
                            ps[:],
                            wp_r[:, db, co * 128 : (co + 1) * 128],
                            yT[:, db, tn * 512 : (tn + 1) * 512],
                            start=(db == 0),
                            stop=(db == DB - 1),
                        )
                    ot = p_ost.tile([128, 512], f32, tag="ot")
                    nc.scalar.activation(ot[:], ps[:], Copy)
                    nc.sync.dma_start(
                        outT_ap[co * 128 : (co + 1) * 128, tn * 512 : (tn + 1) * 512],
                        ot[:],
                    )


def _build_program():
    nc = bacc.Bacc("TRN2", target_bir_lowering=False, debug=False, num_devices=N_CORES)
    aps = {
        "x": nc.dram_tensor("x", [T, C], f32, kind="ExternalInput").ap(),
        "wqk": nc.dram_tensor("wqk", [CB, C, 128], f32, kind="ExternalInput").ap(),
        "wv": nc.dram_tensor("wv", [C, CL], f32, kind="ExternalInput").ap(),
        "wp": nc.dram_tensor("wp", [CL, C], f32, kind="ExternalInput").ap(),
        "bqk": nc.dram_tensor("bqk", [CB, 128], f32, kind="ExternalInput").ap(),
        "masks": nc.dram_tensor("masks", [128, 4, 512], f16, kind="ExternalInput").ap(),
        "id32": nc.dram_tensor("id32", [128, 128], f32, kind="ExternalInput").ap(),
        "outT": nc.dram_tensor("outT", [C, T], f32, kind="ExternalOutput").ap(),
    }
    with tile.TileContext(nc) as tc:
        _emit(tc, aps)
    nc.compile()
    return nc


def get_program():
    if "nc" not in _PROG_CACHE:
        _PROG_CACHE["nc"] = _build_program()
    return _PROG_CACHE["nc"]


def _host_consts():
    j = np.arange(128)[:, None]
    i = np.arange(512)[None, :]
    masks = np.zeros((128, 4, 512), np.float16)
    for m in range(4):
        masks[:, m, :] = (j <= i - 128 * m).astype(np.float16)
    id32 = np.eye(128, dtype=np.float32)
    return masks, id32


def make_in_maps(x, W_attn, b_attn, W_proj):
    """Build the 8 per-core input maps. Core index = 2*batch + head_group."""
    masks, id32 = _host_consts()
    in_maps = []
    for core in range(N_CORES):
        b = core // 2
        g = core % 2
        wq = W_attn[:, g * CL : (g + 1) * CL]
        wk = W_attn[:, C + g * CL : C + (g + 1) * CL]
        wqk = np.stack(
            [wq[:, i * 128 : (i + 1) * 128] for i in range(4)]
            + [wk[:, i * 128 : (i + 1) * 128] for i in range(4)],
            axis=0,
        )  # [8, C, 128]
        wv = W_attn[:, 2 * C + g * CL : 2 * C + (g + 1) * CL]
        bqk = np.concatenate(
            [b_attn[g * CL : (g + 1) * CL], b_attn[C + g * CL : C + (g + 1) * CL]]
        ).reshape(CB, 128)
        in_maps.append(
            {
                "x": np.ascontiguousarray(x[b]),
                "wqk": np.ascontiguousarray(wqk),
                "wv": np.ascontiguousarray(wv),
                "wp": np.ascontiguousarray(W_proj[g * CL : (g + 1) * CL, :]),
                "bqk": np.ascontiguousarray(bqk),
                "masks": masks,
                "id32": id32,
            }
        )
    return in_maps


def run(x, W_attn, b_attn, W_proj, b_proj, trace=False):
    nc = get_program()
    in_maps = make_in_maps(x, W_attn, b_attn, W_proj)
    res = bass_utils.run_bass_kernel_spmd(
        nc, in_maps, core_ids=list(range(N_CORES)), trace=trace
    )
    # combine: out[b] = sum_g outT_{2b+g}^T + (bv_g @ Wp_g summed) + b_proj
    corr = b_proj.astype(np.float64).copy()
    for g in range(2):
        bv_g = b_attn[2 * C + g * CL : 2 * C + (g + 1) * CL]
        corr += bv_g.astype(np.float64) @ W_proj[g * CL : (g + 1) * CL, :].astype(
            np.float64
        )
    out = np.empty((B, T, C), np.float32)
    for b in range(B):
        acc = (
            res.results[2 * b]["outT"].T.astype(np.float64)
            + res.results[2 * b + 1]["outT"].T.astype(np.float64)
            + corr
        )
        out[b] = acc.astype(np.float32)
    return out, res


def kernel(x, W_attn, b_attn, W_proj, b_proj):
    x = np.asarray(x, np.float32)
    W_attn = np.asarray(W_attn, np.float32)
    b_attn = np.asarray(b_attn, np.float32)
    W_proj = np.asarray(W_proj, np.float32)
    b_proj = np.asarray(b_proj, np.float32)
    out, _ = run(x, W_attn, b_attn, W_proj, b_proj)
    return out


# revision 22
# speedup vs baseline: 1.2362x; 1.2362x over previous
"""Causal self-attention (B=4, T=2048, C=1024, H=16) on 8 Trainium2 NeuronCores.

Core index = 2*batch + head_group: each core owns one batch element and 8 of
the 16 heads (tensor-parallel split of c_attn output dim / c_proj input dim).
Each core emits a partial projection out^T [C, T]; the host sums the two
head-group partials per batch and adds the bias terms.

fp16 datapath (fp32 PSUM accumulation everywhere, fp32 softmax denominator):
  x, W_qk, W_v, W_p are cast to fp16 on the host. fp16 weights get FWL
  (fast weight load), making per-matmul LDWEIGHTS ~4x cheaper than fp32/f32r,
  and x^T comes from a single XBAR DMA-transpose instead of 128 PE transposes.

Per-core pipeline (Tile-scheduled, phases overlap via data deps):
  A: xT = DMA-transpose(x)                       [fp16]
  B: qkT[co, tn] = W_qk^T x^T; v = x @ W_v       [fp16 matmuls, fp32 psum]
  C per head h, per 512-wide i-chunk ic:
     S^T[j, i] = k_h^T q_h   (psum groups of 2 j-tiles [128, 2, 512])
     P = exp(S^T / 8)        (one ACT op per group -> fp16)
     causal mask on diagonal groups (DVE, precomputed mask tiles)
     U'^T [65, i] (+)= [v|1]^T P^T  over j-tiles (ones column => rowsum row 64)
     yT[hd, i] = U'^T[0:64] * bcast(1/rowsum)  (ACT copies, gpsimd
                 partition_broadcast, DVE reciprocal + multiply) -> fp16
  D: out^T = W_p^T yT -> fp32 psum -> ACT copy -> DMA
"""

import numpy as np

import concourse.bass as bass
import concourse.mybir as mybir
import concourse.tile as tile
from concourse import bacc, bass_utils

B, T, C, H = 4, 2048, 1024, 16
HD = C // H          # 64 head dim
N_CORES = 8
HG = H // 2          # 8 heads per core
CL = HG * HD         # 512 local width of q/k/v
TT = T // 128        # 16 t-tiles
CB = C // 128        # 8 c-tiles
DB = CL // 128       # 4 local-hd tiles
NIC = T // 512       # i-chunks (4)

f32 = mybir.dt.float32
f16 = mybir.dt.float16

_PROG_CACHE = {}


def _emit(tc, aps):
    nc = tc.nc
    Exp = mybir.ActivationFunctionType.Exp
    Copy = mybir.ActivationFunctionType.Copy

    x_ap = aps["x"]
    wqk_ap = aps["wqk"]
    wv_ap = aps["wv"]
    wp_ap = aps["wp"]
    bqk_ap = aps["bqk"]
    masks_ap = aps["masks"]
    outT_ap = aps["outT"]

    from contextlib import ExitStack

    with ExitStack() as outer:
        const = outer.enter_context(tc.tile_pool(name="const", bufs=1))
        p_xT = outer.enter_context(tc.tile_pool(name="xT", bufs=1))
        p_qkT = outer.enter_context(tc.tile_pool(name="qkT", bufs=1))
        p_v = outer.enter_context(tc.tile_pool(name="vv", bufs=1))
        p_yT = outer.enter_context(tc.tile_pool(name="yT", bufs=1))
        p_w = outer.enter_context(tc.tile_pool(name="wsb", bufs=1))

        masks = const.tile([128, 4, 512], f16)
        nc.sync.dma_start(masks[:], masks_ap)
        bqk = const.tile([128, CB], f32)
        nc.sync.dma_start(bqk[:], bqk_ap.rearrange("co p -> p co"))

        # A: x^T in one XBAR DMA-transpose
        xT = p_xT.tile([128, CB, T], f16)
        nc.sync.dma_start_transpose(xT[:], x_ap[:])

        # weights resident (fp16, no rounding passes)
        wqk_sb = p_w.tile([128, CB, CB * 128], f16)  # [c-part, cb, co*128+q]
        nc.sync.dma_start(wqk_sb[:], wqk_ap.rearrange("(cb p) n -> p cb n", p=128))
        wv_sb = p_w.tile([128, CB, CL], f16)
        nc.sync.dma_start(wv_sb[:], wv_ap.rearrange("(cb p) n -> p cb n", p=128))

        # per-(co, tn) qkT tiles for fine-grained deps
        qkT = {}
        for co in range(CB):
            for tn in range(NIC):
                qkT[(co, tn)] = p_qkT.tile(
                    [128, 512], f16, tag=f"qkT_{co}_{tn}", name=f"qkT_{co}_{tn}"
                )
        # per-jt v' tiles [j-part, head, v(64) | ones]
        vv = {}
        for jt in range(TT):
            vv[jt] = p_v.tile([128, HG, HD + 1], f16, tag=f"vv_{jt}", name=f"vv_{jt}")
            nc.vector.memset(vv[jt][:, :, HD : HD + 1], 1.0)
        yT = p_yT.tile([128, DB, T], f16)

        with ExitStack() as s_ab:
            ps_ab = ExitStack()
            ps_mm = ps_ab.enter_context(tc.tile_pool(name="ps_mm", bufs=4, space="PSUM"))

            # B-qk
            for co in range(CB):
                for tn in range(NIC):
                    ps = ps_mm.tile([128, 512], f32, tag="mm")
                    for cb in range(CB):
                        nc.tensor.matmul(
                            ps[:],
                            wqk_sb[:, cb, co * 128 : (co + 1) * 128],
                            xT[:, cb, tn * 512 : (tn + 1) * 512],
                            start=(cb == 0),
                            stop=(cb == CB - 1),
                        )
                    nc.vector.tensor_scalar_add(qkT[(co, tn)][:], ps[:], bqk[:, co : co + 1])
            # B-v
            for tt in range(TT):
                ps = ps_mm.tile([128, CL], f32, tag="mm")
                for cb in range(CB):
                    nc.tensor.matmul(
                        ps[:],
                        xT[:, cb, tt * 128 : (tt + 1) * 128],
                        wv_sb[:, cb, :],
                        start=(cb == 0),
                        stop=(cb == CB - 1),
                    )
                nc.vector.tensor_copy(
                    vv[tt][:, :, 0:HD], ps.rearrange("p (h d) -> p h d", d=HD)
                )

            ps_ab.close()  # free A/B psum banks (per-tile release deps)

            # ---- attention ---------------------------------------------------
            with ExitStack() as s_c:
                p_p = s_c.enter_context(tc.tile_pool(name="pp", bufs=12))
                p_usb = s_c.enter_context(tc.tile_pool(name="usb", bufs=3))
                p_rb = s_c.enter_context(tc.tile_pool(name="rb", bufs=3))
                ps_sc = s_c.enter_context(tc.tile_pool(name="ps_sc", bufs=3, space="PSUM"))
                ps_u = s_c.enter_context(tc.tile_pool(name="ps_u", bufs=2, space="PSUM"))

                for h in range(HG):
                    poff = 64 * (h % 2)
                    co_q = h // 2
                    co_k = 4 + h // 2
                    for ic in range(NIC):
                        n_jt = 4 * (ic + 1)
                        n_g = n_jt // 2
                        ptiles = []
                        for g in range(n_g):
                            psg = ps_sc.tile([128, 2, 512], f32, tag="sc")
                            for ss in range(2):
                                jt = 2 * g + ss
                                nc.tensor.matmul(
                                    psg[:, ss, :],
                                    qkT[(co_k, jt // 4)][
                                        poff : poff + 64, (jt % 4) * 128 : (jt % 4 + 1) * 128
                                    ],
                                    qkT[(co_q, ic)][poff : poff + 64, :],
                                    start=True,
                                    stop=True,
                                )
                            pt = p_p.tile([128, 2, 512], f16, tag="p")
                            nc.scalar.activation(pt[:], psg[:], Exp, scale=1.0 / np.sqrt(HD))
                            if 2 * g + 1 >= 4 * ic:  # group contains diagonal blocks
                                m0 = 2 * g - 4 * ic
                                nc.vector.tensor_mul(pt[:], pt[:], masks[:, m0 : m0 + 2, :])
                            ptiles.append(pt)
                        up = ps_u.tile([HD + 1, 512], f32, tag="u")
                        for jt in range(n_jt):
                            nc.tensor.matmul(
                                up[:],
                                vv[jt][:, h, :],
                                ptiles[jt // 2][:, jt % 2, :],
                                start=(jt == 0),
                                stop=(jt == n_jt - 1),
                            )
                        usb = p_usb.tile([HD, 512], f32, tag="usb")
                        nc.scalar.activation(usb[:], up[0:HD, :], Copy)
                        rs = p_rb.tile([1, 512], f32, tag="rs")
                        nc.scalar.activation(rs[:], up[HD : HD + 1, :], Copy)
                        rb = p_rb.tile([HD, 512], f32, tag="rb")
                        nc.gpsimd.partition_broadcast(rb[:], rs[0:1, :], channels=HD)
                        nc.vector.reciprocal(rb[:], rb[:])
                        nc.vector.tensor_mul(
                            yT[poff : poff + HD, h // 2, ic * 512 : (ic + 1) * 512],
                            usb[:],
                            rb[:],
                        )

        # ---- projection ------------------------------------------------------
        with ExitStack() as s_d:
            p_wp = s_d.enter_context(tc.tile_pool(name="wp", bufs=1))
            p_ost = s_d.enter_context(tc.tile_pool(name="ost", bufs=4))
            ps_pj = s_d.enter_context(tc.tile_pool(name="ps_pj", bufs=4, space="PSUM"))

            wp_sb = p_wp.tile([128, DB, C], f16)
            nc.sync.dma_start(wp_sb[:], wp_ap.rearrange("(db p) c -> p db c", p=128))

            for co in range(CB):
                for tn in range(NIC):
                    ps = ps_pj.tile([128, 512], f32, tag="pj")
                    for db in range(DB):
                        nc.tensor.matmul(
                            ps[:],
                            wp_sb[:, db, co * 128 : (co + 1) * 128],
                            yT[:, db, tn * 512 : (tn + 1) * 512],
                            start=(db == 0),
                            stop=(db == DB - 1),
                        )
                    ot = p_ost.tile([128, 512], f32, tag="ot")
                    nc.scalar.activation(ot[:], ps[:], Copy)
                    nc.sync.dma_start(
                        outT_ap[co * 128 : (co + 1) * 128, tn * 512 : (tn + 1) * 512],
                        ot[:],
                    )


def _build_program():
    nc = bacc.Bacc("TRN2", target_bir_lowering=False, debug=False, num_devices=N_CORES)
    aps = {
        "x": nc.dram_tensor("x", [T, C], f16, kind="ExternalInput").ap(),
        "wqk": nc.dram_tensor("wqk", [C, CB * 128], f16, kind="ExternalInput").ap(),
        "wv": nc.dram_tensor("wv", [C, CL], f16, kind="ExternalInput").ap(),
        "wp": nc.dram_tensor("wp", [CL, C], f16, kind="ExternalInput").ap(),
        "bqk": nc.dram_tensor("bqk", [CB, 128], f32, kind="ExternalInput").ap(),
        "masks": nc.dram_tensor("masks", [128, 4, 512], f16, kind="ExternalInput").ap(),
        "outT": nc.dram_tensor("outT", [C, T], f32, kind="ExternalOutput").ap(),
    }
    with tile.TileContext(nc) as tc:
        _emit(tc, aps)
    nc.compile()
    return nc


def get_program():
    if "nc" not in _PROG_CACHE:
        _PROG_CACHE["nc"] = _build_program()
    return _PROG_CACHE["nc"]


def _host_consts():
    j = np.arange(128)[:, None]
    i = np.arange(512)[None, :]
    masks = np.zeros((128, 4, 512), np.float16)
    for m in range(4):
        masks[:, m, :] = (j <= i - 128 * m).astype(np.float16)
    return masks


def make_in_maps(x, W_attn, b_attn, W_proj):
    """Build the 8 per-core input maps. Core index = 2*batch + head_group."""
    masks = _host_consts()
    in_maps = []
    for core in range(N_CORES):
        b = core // 2
        g = core % 2
        wq = W_attn[:, g * CL : (g + 1) * CL]
        wk = W_attn[:, C + g * CL : C + (g + 1) * CL]
        wqk = np.concatenate([wq, wk], axis=1)  # [C, 1024], cols = co*128+q
        wv = W_attn[:, 2 * C + g * CL : 2 * C + (g + 1) * CL]
        bqk = np.concatenate(
            [b_attn[g * CL : (g + 1) * CL], b_attn[C + g * CL : C + (g + 1) * CL]]
        ).reshape(CB, 128)
        in_maps.append(
            {
                "x": np.ascontiguousarray(x[b]).astype(np.float16),
                "wqk": np.ascontiguousarray(wqk).astype(np.float16),
                "wv": np.ascontiguousarray(wv).astype(np.float16),
                "wp": np.ascontiguousarray(W_proj[g * CL : (g + 1) * CL, :]).astype(
                    np.float16
                ),
                "bqk": np.ascontiguousarray(bqk).astype(np.float32),
                "masks": masks,
            }
        )
    return in_maps


def run(x, W_attn, b_attn, W_proj, b_proj, trace=False):
    nc = get_program()
    in_maps = make_in_maps(x, W_attn, b_attn, W_proj)
    res = bass_utils.run_bass_kernel_spmd(
        nc, in_maps, core_ids=list(range(N_CORES)), trace=trace
    )
    # combine: out[b] = sum_g outT_{2b+g}^T + (bv_g @ Wp_g summed) + b_proj
    corr = b_proj.astype(np.float64).copy()
    for g in range(2):
        bv_g = b_attn[2 * C + g * CL : 2 * C + (g + 1) * CL]
        corr += bv_g.astype(np.float64) @ W_proj[g * CL : (g + 1) * CL, :].astype(
            np.float64
        )
    out = np.empty((B, T, C), np.float32)
    for b in range(B):
        acc = (
            res.results[2 * b]["outT"].T.astype(np.float64)
            + res.results[2 * b + 1]["outT"].T.astype(np.float64)
            + corr
        )
        out[b] = acc.astype(np.float32)
    return out, res


def kernel(x, W_attn, b_attn, W_proj, b_proj):
    x = np.asarray(x, np.float32)
    W_attn = np.asarray(W_attn, np.float32)
    b_attn = np.asarray(b_attn, np.float32)
    W_proj = np.asarray(W_proj, np.float32)
    b_proj = np.asarray(b_proj, np.float32)
    out, _ = run(x, W_attn, b_attn, W_proj, b_proj)
    return out
